# revision 20
# baseline (speedup 1.0000x reference)
"""GCN message-passing kernel for Trainium2, 8-core SPMD.

Model (N=8192 nodes, 64 graphs of 128 consecutive nodes):
  h   = emb[x]
  h   = GCN layer 1:  D_r^-1/2 m D_c^-1/2 relu(h W1^T + b1)
  h   = GCN layer 2:  D_r^-1/2 m D_c^-1/2 relu(h W2^T + b2)
  out = segment_max(h, 128-row blocks) @ Wc^T + bc

Distribution & dataflow:
  - m is row-sharded across the 8 cores. The host ships each core its
    shard already transposed to [j, i] tile layout [128, 64, 1024] and
    cast to fp8e4m3 (8 MB of HBM traffic per core instead of 32).
  - The host also performs the embedding row gather (pure data
    movement) and ships h^T replicated as bf16 [128, 64, 128]; the
    embedding table itself never hits the device.
  - Column-degree partials overlap the m DMA: half the j-tiles are
    free-axis reduce_sum on DVE, half ride scalar-engine copies via
    activation accum_out. One ReduceScatter+AllGather produces
    s_c = rsqrt(col_deg) (full) and the local slice.
  - msg1 = relu(h W1^T + b1) is computed unscaled during the load;
    after the collective it is scaled by 64*s_c into fp8 (64 shifts
    the values into e4m3's normal range), one tile ahead of the
    layer-1 matmul, which runs fp8 DoubleRow against resident mT.
    A ones(*64) column in msg yields row degrees (s_r) for free.
  - msg2 = 64*s_c*relu(s_r*(t1 W2^T) + ...) is fp8, AllGathered as
    1 MB; layer-2 is a msg-stationary fp8 DoubleRow matmul
    accumulating h2^T [f, i] in two PSUM banks. The (s_r/64) scaling
    is a gpsimd partition-broadcast plus one DVE multiply per half
    (the /64 undoes both fp8 range shifts); pooling is a single
    strided reduce_max off h2^T.
"""

import sys

for p in ("/opt/trn_rl_repo",):
    if p not in sys.path:
        sys.path.insert(0, p)

from contextlib import ExitStack

import numpy as np

import concourse.bass as bass
import concourse.mybir as mybir
import concourse.tile as tile
from concourse import bacc, bass_utils
from concourse.masks import make_identity

P = 128
N = 8192
NCORES = 8
NS = N // NCORES          # rows per core (1024)
JT = N // P               # j tiles (64)
IB = NS // P              # i blocks per core (8)
F = 128                   # hidden/emb width
C = 16                    # classes
G_LOCAL = IB              # graphs per core (graph == one 128-row block)
MSG_SCALE = 64.0          # fp8 range shift for msg1/msg2, undone in s_r mult
USE_DOUBLE_ROW = True     # fp8 DoubleRow for the two big matmuls

F32 = mybir.dt.float32
BF16 = mybir.dt.bfloat16
F8 = mybir.dt.float8e4

M_NP_DTYPE = mybir.dt.np(F8)
BF16_NP = mybir.dt.np(BF16)

_CACHE = {}


def _build(reps=1):
    nc = bacc.Bacc("TRN2", target_bir_lowering=False, debug=False,
                   enable_asserts=True, num_devices=NCORES)

    mT_in = nc.dram_tensor("mT_in", [P, JT, NS], F8, kind="ExternalInput")
    hT_in = nc.dram_tensor("hT_in", [P, JT, F], BF16, kind="ExternalInput")
    w1_in = nc.dram_tensor("w1_in", [F, F], F32, kind="ExternalInput")
    b1_in = nc.dram_tensor("b1_in", [F], F32, kind="ExternalInput")
    w2_in = nc.dram_tensor("w2_in", [F, F], F32, kind="ExternalInput")
    b2_in = nc.dram_tensor("b2_in", [F], F32, kind="ExternalInput")
    wc_in = nc.dram_tensor("wc_in", [C, F], F32, kind="ExternalInput")
    bc_in = nc.dram_tensor("bc_in", [C], F32, kind="ExternalInput")
    out_l = nc.dram_tensor("out_l", [G_LOCAL, C], F32, kind="ExternalOutput")

    with tile.TileContext(nc) as tc, ExitStack() as stack:
        consts = stack.enter_context(tc.tile_pool(name="consts", bufs=1))
        big = stack.enter_context(tc.tile_pool(name="big", bufs=1))
        dram = stack.enter_context(tc.tile_pool(name="dram", bufs=1, space="DRAM"))

        ident_bf = consts.tile([P, P], BF16)
        make_identity(nc, ident_bf)
        ident_f32 = consts.tile([P, P], F32)
        make_identity(nc, ident_f32)

        # ---- small constants -------------------------------------------
        ones_row = consts.tile([1, P], BF16)
        nc.vector.memset(ones_row[:], 1.0)
        ones_row8_f32 = consts.tile([1, G_LOCAL], F32)
        nc.vector.memset(ones_row8_f32[:], 1.0)
        b1_row = consts.tile([1, F], BF16)
        nc.gpsimd.dma_start(b1_row[:], b1_in.ap()[None, :])
        b2_row = consts.tile([1, F], BF16)
        nc.gpsimd.dma_start(b2_row[:], b2_in.ap()[None, :])
        bc_row = consts.tile([1, C], F32)
        nc.sync.dma_start(bc_row[:], bc_in.ap()[None, :])

        # w1T/w2T (transposed weights, bf16), wcT (f32)
        w1T = consts.tile([P, F], BF16)
        w2T = consts.tile([P, F], BF16)
        wcT = consts.tile([P, C], F32)
        with tc.tile_pool(name="wtmp", bufs=2) as wtmp, \
             tc.tile_pool(name="wpsum", bufs=2, space="PSUM") as wpsum:
            for w_in, wT in ((w1_in, w1T), (w2_in, w2T)):
                wf = wtmp.tile([F, F], F32, tag="wf")
                nc.sync.dma_start(wf[:], w_in.ap())
                wb = wtmp.tile([F, F], BF16, tag="wb")
                nc.vector.tensor_copy(wb[:], wf[:])
                ps = wpsum.tile([P, F], BF16, tag="wps")
                nc.tensor.transpose(ps[:], wb[:], ident_bf[:])
                nc.any.tensor_copy(wT[:], ps[:])
            wcf = wtmp.tile([C, F], F32, tag="wcf")
            nc.sync.dma_start(wcf[:], wc_in.ap())
            pc = wpsum.tile([P, C], F32, tag="wcps")
            nc.tensor.transpose(pc[:], wcf[:], ident_f32[:C, :C])
            nc.any.tensor_copy(wcT[:], pc[:])

        for _rep in range(reps):
            _emit_pipeline(
                nc, tc, consts, big, dram,
                mT_in, hT_in, out_l,
                ident_bf, ident_f32, ones_row, ones_row8_f32,
                b1_row, b2_row, bc_row, w1T, w2T, wcT,
            )

    nc.compile()
    return nc


def _emit_pipeline(nc, tc, consts, big, dram, mT_in, hT_in, out_l,
                   ident_bf, ident_f32, ones_row, ones_row8_f32,
                   b1_row, b2_row, bc_row, w1T, w2T, wcT):
    # ---- resident tensors ------------------------------------------
    mT = big.tile([P, JT, NS], F8, tag="mT", name="mT")          # [j_in_tile, jt, i]
    hT = big.tile([P, JT, F], BF16, tag="hT", name="hT")         # [e, jt, j_in_tile]
    msg_r = big.tile([P, JT, F], BF16, tag="msg_r", name="msg_r")   # relu, unscaled
    msg_f8 = big.tile([P, JT, F + 1], F8, tag="msg", name="msg")    # 64*sc*msg1 | 64
    ones_bf_scr = consts.tile([P, JT], BF16, tag="ones_scr", name="ones_scr")
    nc.vector.memset(ones_bf_scr[:], MSG_SCALE)
    nc.vector.tensor_copy(msg_f8[:, :, F], ones_bf_scr[:])
    cd_acc = big.tile([P, JT], F32, tag="cd_acc", name="cd_acc")

    # ---- phase A: mT + hT DMA; cd partials; msg1 relu (unscaled) ----
    nc.sync.dma_start(hT[:], hT_in.ap())
    KC = 8  # j-tiles per mT DMA chunk
    with tc.tile_pool(name="cdscratch", bufs=2) as cds, \
         tc.tile_pool(name="mpsum", bufs=3, space="PSUM") as mpsum:
        for k in range(JT // KC):
            nc.sync.dma_start(mT[:, k * KC:(k + 1) * KC, :],
                              mT_in.ap()[:, k * KC:(k + 1) * KC, :])
            for jt in range(k * KC, (k + 1) * KC):
                # cd partial: DVE reduce (~1.07us) / ACT accum-copy (~0.73us);
                # ACT gets the larger share so both drain together
                if jt % 8 in (0, 3, 6):
                    nc.vector.reduce_sum(
                        out=cd_acc[:, jt:jt + 1], in_=mT[:, jt, :],
                        axis=mybir.AxisListType.X)
                else:
                    scr = cds.tile([P, NS], F8, tag="cds", name="cds")
                    nc.scalar.activation(
                        scr[:], mT[:, jt, :],
                        mybir.ActivationFunctionType.Copy,
                        accum_out=cd_acc[:, jt:jt + 1])
            # msg1 pre-relu staging for this chunk (PE matmuls + DVE/ACT
            # copies); relu rides the post-collective scale (relu(s*z)=s*relu(z))
            for jt in range(k * KC, (k + 1) * KC):
                mps = mpsum.tile([P, F], F32, tag="mps", name="mps")
                nc.tensor.matmul(mps[:], hT[:, jt, :], w1T[:], start=True, stop=False)
                nc.tensor.matmul(mps[:], ones_row[:], b1_row[:], start=False, stop=True)
                if jt % 16 < 11:
                    nc.vector.tensor_copy(msg_r[:, jt, :], mps[:])
                else:
                    nc.scalar.activation(msg_r[:, jt, :], mps[:],
                                         mybir.ActivationFunctionType.Copy)

    # ---- column-degree collectives ---------------------------------
    cd_part = dram.tile([N], F32, tag="cd_part", name="cd_part")
    cd_loc = dram.tile([NS], F32, tag="cd_loc", name="cd_loc")
    cd_full = dram.tile([N], F32, tag="cd_full", name="cd_full", addr_space="Shared")
    nc.sync.dma_start(cd_part[:].rearrange("(t p) -> p t", p=P), cd_acc[:])
    nc.gpsimd.collective_compute(
        "ReduceScatter", mybir.AluOpType.add,
        replica_groups=[list(range(NCORES))],
        ins=[cd_part.opt()], outs=[cd_loc.opt()],
    )
    nc.gpsimd.collective_compute(
        "AllGather", mybir.AluOpType.bypass,
        replica_groups=[list(range(NCORES))],
        ins=[cd_loc.opt()], outs=[cd_full.opt()],
    )
    cd_full_sb = consts.tile([P, JT], F32, tag="cdf_sb", name="cdf_sb")
    nc.sync.dma_start(cd_full_sb[:], cd_full[:].rearrange("(t p) -> p t", p=P))
    cd_loc_sb = consts.tile([P, IB], F32, tag="cdl_sb", name="cdl_sb")
    nc.sync.dma_start(cd_loc_sb[:], cd_loc[:].rearrange("(b p) -> p b", p=P))

    s_c64 = consts.tile([P, JT], F32, tag="s_c64", name="s_c64")
    nc.scalar.sqrt(s_c64[:], cd_full_sb[:])
    nc.vector.reciprocal(s_c64[:], s_c64[:])
    nc.vector.tensor_scalar_mul(s_c64[:], s_c64[:], MSG_SCALE)
    s_c_loc64 = consts.tile([P, IB], F32, tag="s_c_loc64", name="s_c_loc64")
    nc.scalar.sqrt(s_c_loc64[:], cd_loc_sb[:])
    nc.vector.reciprocal(s_c_loc64[:], s_c_loc64[:])
    nc.vector.tensor_scalar_mul(s_c_loc64[:], s_c_loc64[:], MSG_SCALE)

    # ---- phase C: t1*64 = m @ [64*s_c*msg1|64]; scaling pipelined ---
    s_r = consts.tile([P, IB], F32, tag="s_r", name="s_r")
    s_r64 = consts.tile([P, IB], F32, tag="s_r64", name="s_r64")
    h1_bf = consts.tile([P, IB, F], BF16, tag="h1_bf", name="h1_bf")
    with tc.tile_pool(name="c_psum", bufs=1, space="PSUM") as cpsum:
        pt1 = [cpsum.tile([P, F + 1], F32, tag=f"t1_{b}", name=f"t1_{b}")
               for b in range(IB)]
        def emit_scale(jt):
            # msg = relu(s_c64 * z) into fp8, split across ACT/DVE/gpsimd
            if jt % 3 == 0:
                nc.scalar.activation(
                    msg_f8[:, jt, 0:F], msg_r[:, jt, :],
                    mybir.ActivationFunctionType.Relu,
                    scale=s_c64[:, jt:jt + 1],
                )
            elif jt % 3 == 1:
                nc.vector.tensor_scalar(
                    out=msg_f8[:, jt, 0:F], in0=msg_r[:, jt, :],
                    scalar1=s_c64[:, jt:jt + 1], scalar2=0.0,
                    op0=mybir.AluOpType.mult, op1=mybir.AluOpType.max)
            else:
                nc.gpsimd.tensor_scalar(
                    out=msg_f8[:, jt, 0:F], in0=msg_r[:, jt, :],
                    scalar1=s_c64[:, jt:jt + 1], scalar2=0.0,
                    op0=mybir.AluOpType.mult, op1=mybir.AluOpType.max)

        # all scales first (3-way engine split, ~8us wall) so the PE
        # DoubleRow stream then runs back-to-back at full p-state
        for jt in range(JT):
            emit_scale(jt)
        if USE_DOUBLE_ROW:
            for tp in range(JT // 2):
                for b in range(IB):
                    nc.tensor.matmul(
                        pt1[b][:],
                        mT[:, 2 * tp:2 * tp + 2, b * P:(b + 1) * P],
                        msg_f8[:, 2 * tp:2 * tp + 2, :],
                        start=(tp == 0), stop=(tp == JT // 2 - 1),
                        perf_mode=mybir.MatmulPerfMode.DoubleRow,
                    )
        else:
            for jt in range(JT):
                for b in range(IB):
                    nc.tensor.matmul(
                        pt1[b][:], mT[:, jt, b * P:(b + 1) * P], msg_f8[:, jt, :],
                        start=(jt == 0), stop=(jt == JT - 1),
                    )
        # s_r = rsqrt(rd), rd = col F / 64;  h1 = (s_r/64) * t1*64
        for b in range(IB):
            nc.vector.tensor_scalar_mul(s_r[:, b:b + 1], pt1[b][:, F:F + 1],
                                        1.0 / MSG_SCALE)
        nc.scalar.sqrt(s_r[:], s_r[:])
        nc.vector.reciprocal(s_r[:], s_r[:])
        nc.vector.tensor_scalar_mul(s_r64[:], s_r[:], 1.0 / MSG_SCALE)
        for b in range(IB):
            nc.scalar.activation(
                h1_bf[:, b, :], pt1[b][:, 0:F],
                mybir.ActivationFunctionType.Copy,
                scale=s_r64[:, b:b + 1],
            )

    # s_r/64 as a [1, NS] row via a dram round-trip + gpsimd partition
    # broadcast — issued here so it hides under phase D and the AllGather
    srd = dram.tile([NS], F32, tag="srd", name="srd")
    nc.sync.dma_start(srd[:].rearrange("(b p) -> p b", p=P), s_r64[:])
    s_r_row = consts.tile([1, NS], F32, tag="s_r_row", name="s_r_row")
    nc.sync.dma_start(s_r_row[:], srd[:][None, :])
    srb_sb = consts.tile([P, NS], F32, tag="srb_sb", name="srb_sb")
    nc.gpsimd.partition_broadcast(srb_sb[:], s_r_row[:])

    # ---- phase D: msg2 = 64*sc*relu(h1 W2^T + b2) as fp8, AllGather -
    msg2_sb = consts.tile([P, IB, F], F8, tag="msg2", name="msg2")
    with tc.tile_pool(name="d_work", bufs=2) as dwork, \
         tc.tile_pool(name="d_psum", bufs=2, space="PSUM") as dpsum, \
         tc.tile_pool(name="d_tpsum", bufs=2, space="PSUM") as dtpsum:
        for b in range(IB):
            tps = dtpsum.tile([P, P], BF16, tag="dtps", name="dtps")
            nc.tensor.transpose(tps[:], h1_bf[:, b, :], ident_bf[:])
            h1T = dwork.tile([P, F], BF16, tag="h1T", name="h1T")
            nc.any.tensor_copy(h1T[:], tps[:])
            ps = dpsum.tile([P, F], F32, tag="dps", name="dps")
            nc.tensor.matmul(ps[:], h1T[:], w2T[:], start=True, stop=False)
            nc.tensor.matmul(ps[:], ones_row[:], b2_row[:], start=False, stop=True)
            nc.scalar.activation(
                msg2_sb[:, b, :], ps[:],
                mybir.ActivationFunctionType.Relu,
                scale=s_c_loc64[:, b:b + 1],
            )

    msg2_loc = dram.tile([NS, F], F8, tag="m2l", name="m2l")
    msg2_full = dram.tile([N, F], F8, tag="m2f", name="m2f", addr_space="Shared")
    nc.sync.dma_start(
        msg2_loc[:].rearrange("(b p) g -> p b g", p=P), msg2_sb[:])
    nc.gpsimd.collective_compute(
        "AllGather", mybir.AluOpType.bypass,
        replica_groups=[list(range(NCORES))],
        ins=[msg2_loc.opt()], outs=[msg2_full.opt()],
    )
    m2f_sb = big.tile([P, JT, F], F8, tag="m2f_sb", name="m2f_sb")
    RB = 16  # j-tiles per msg2_full readback chunk (overlaps phase E)
    for k in range(JT // RB):
        nc.sync.dma_start(
            m2f_sb[:, k * RB:(k + 1) * RB, :],
            msg2_full[:].rearrange("(t p) g -> p t g", p=P)[:, k * RB:(k + 1) * RB, :])

    # ---- phase E: h2^T = (s_r/64) * (msg2^T m)^T, msg-stationary ----
    HNS = NS // 2
    h2T = consts.tile([P, NS], F32, tag="h2T", name="h2T")
    with tc.tile_pool(name="e_psum", bufs=1, space="PSUM") as epsum:
        pe = [epsum.tile([P, HNS], F32, tag=f"t2_{h}", name=f"t2_{h}")
              for h in range(2)]
        if USE_DOUBLE_ROW:
            for tp in range(JT // 2):
                for h in range(2):
                    nc.tensor.matmul(
                        pe[h][:],
                        m2f_sb[:, 2 * tp:2 * tp + 2, :],
                        mT[:, 2 * tp:2 * tp + 2, h * HNS:(h + 1) * HNS],
                        start=(tp == 0), stop=(tp == JT // 2 - 1),
                        perf_mode=mybir.MatmulPerfMode.DoubleRow,
                    )
        else:
            for jt in range(JT):
                for h in range(2):
                    nc.tensor.matmul(
                        pe[h][:], m2f_sb[:, jt, :],
                        mT[:, jt, h * HNS:(h + 1) * HNS],
                        start=(jt == 0), stop=(jt == JT - 1),
                    )
        for h in range(2):
            nc.vector.scalar_tensor_tensor(
                out=h2T[:, h * HNS:(h + 1) * HNS],
                in0=pe[h][:], scalar=1.0, in1=srb_sb[:, h * HNS:(h + 1) * HNS],
                op0=mybir.AluOpType.mult, op1=mybir.AluOpType.mult,
            )

    # ---- phase F: segment max + classifier -------------------------
    pooledT = consts.tile([P, G_LOCAL], F32, tag="pooledT", name="pooledT")
    out_sb = consts.tile([G_LOCAL, C], F32, tag="out_sb", name="out_sb")
    nc.vector.reduce_max(
        out=pooledT[:], in_=h2T[:].rearrange("p (g q) -> p g q", q=P),
        axis=mybir.AxisListType.X)
    with tc.tile_pool(name="cls_psum", bufs=1, space="PSUM") as clspsum:
        cps = clspsum.tile([G_LOCAL, C], F32, tag="cls", name="cls")
        nc.tensor.matmul(cps[:], pooledT[:], wcT[:], start=True, stop=False)
        nc.tensor.matmul(cps[:], ones_row8_f32[:], bc_row[:],
                         start=False, stop=True)
        nc.vector.tensor_copy(out_sb[:], cps[:])
    nc.sync.dma_start(out_l.ap(), out_sb[:])


def _get_nc():
    if "nc" not in _CACHE:
        _CACHE["nc"] = _build()
    return _CACHE["nc"]


def make_in_maps(inputs):
    m = np.asarray(inputs["m"], dtype=np.float32)
    x = np.asarray(inputs["x"]).astype(np.int64)
    emb = np.asarray(inputs["emb"], dtype=np.float32)
    w1 = np.ascontiguousarray(np.asarray(inputs["w1"], dtype=np.float32))
    b1 = np.ascontiguousarray(np.asarray(inputs["b1"], dtype=np.float32))
    w2 = np.ascontiguousarray(np.asarray(inputs["w2"], dtype=np.float32))
    b2 = np.ascontiguousarray(np.asarray(inputs["b2"], dtype=np.float32))
    wc = np.ascontiguousarray(np.asarray(inputs["wc"], dtype=np.float32))
    bc = np.ascontiguousarray(np.asarray(inputs["bc"], dtype=np.float32))

    # host-side layout prep: embedding gather + per-core transposes
    h = emb[x]                                   # (N, F) float32
    hT = np.ascontiguousarray(
        h.reshape(JT, P, F).transpose(2, 0, 1)).astype(BF16_NP)  # [e, t, p]
    m8 = m.astype(M_NP_DTYPE)                    # fp8 cast once
    m8v = m8.reshape(N, JT, P)                   # [i_glob, t, p]

    in_maps = []
    for k in range(NCORES):
        blk = m8v[k * NS:(k + 1) * NS]           # [i_loc, t, p]
        mT = np.ascontiguousarray(blk.transpose(2, 1, 0))  # [p, t, i_loc]
        in_maps.append({
            "mT_in": mT, "hT_in": hT,
            "w1_in": w1, "b1_in": b1, "w2_in": w2, "b2_in": b2,
            "wc_in": wc, "bc_in": bc,
        })
    return in_maps


def kernel(**inputs):
    nc = _get_nc()
    in_maps = make_in_maps(inputs)
    res = bass_utils.run_bass_kernel_spmd(
        nc, in_maps, core_ids=list(range(NCORES)))
    out = np.concatenate([res.results[k]["out_l"] for k in range(NCORES)], axis=0)
    return out.astype(np.float32)


# revision 27
# speedup vs baseline: 1.1549x; 1.1549x over previous
"""GCN message-passing kernel for Trainium2, 8-core SPMD.

Model (N=8192 nodes, 64 graphs of 128 consecutive nodes):
  h   = emb[x]
  h   = GCN layer 1:  D_r^-1/2 m D_c^-1/2 relu(h W1^T + b1)
  h   = GCN layer 2:  D_r^-1/2 m D_c^-1/2 relu(h W2^T + b2)
  out = segment_max(h, 128-row blocks) @ Wc^T + bc

Distribution & dataflow:
  - m is row-sharded across the 8 cores. The host ships each core its
    shard already transposed to [j, i] tile layout [128, 64, 1024] and
    cast to fp8e4m3 (8 MB of HBM traffic per core instead of 32).
  - The host also performs the embedding row gather (pure data
    movement) and ships h^T replicated as bf16 [128, 64, 128]; the
    embedding table itself never hits the device.
  - Column-degree partials overlap the m DMA: half the j-tiles are
    free-axis reduce_sum on DVE, half ride scalar-engine copies via
    activation accum_out. One ReduceScatter+AllGather produces
    s_c = rsqrt(col_deg) (full) and the local slice.
  - msg1 = relu(h W1^T + b1) is computed unscaled during the load;
    after the collective it is scaled by 64*s_c into fp8 (64 shifts
    the values into e4m3's normal range), one tile ahead of the
    layer-1 matmul, which runs fp8 DoubleRow against resident mT.
    A ones(*64) column in msg yields row degrees (s_r) for free.
  - msg2 = 64*s_c*relu(s_r*(t1 W2^T) + ...) is fp8, AllGathered as
    1 MB; layer-2 is a msg-stationary fp8 DoubleRow matmul
    accumulating h2^T [f, i] in two PSUM banks. The (s_r/64) scaling
    is a gpsimd partition-broadcast plus one DVE multiply per half
    (the /64 undoes both fp8 range shifts); pooling is a single
    strided reduce_max off h2^T.
"""

import sys

for p in ("/opt/trn_rl_repo",):
    if p not in sys.path:
        sys.path.insert(0, p)

from contextlib import ExitStack

import numpy as np

import concourse.bass as bass
import concourse.mybir as mybir
import concourse.tile as tile
from concourse import bacc, bass_utils
from concourse.masks import make_identity

P = 128
N = 8192
NCORES = 8
NS = N // NCORES          # rows per core (1024)
JT = N // P               # j tiles (64)
IB = NS // P              # i blocks per core (8)
F = 128                   # hidden/emb width
C = 16                    # classes
G_LOCAL = IB              # graphs per core (graph == one 128-row block)
MSG_SCALE = 64.0          # fp8 range shift for msg1/msg2, undone in s_r mult
USE_DOUBLE_ROW = True     # fp8 DoubleRow for the two big matmuls

F32 = mybir.dt.float32
BF16 = mybir.dt.bfloat16
F8 = mybir.dt.float8e4

M_NP_DTYPE = mybir.dt.np(F8)
BF16_NP = mybir.dt.np(BF16)

_CACHE = {}


def _build(reps=1):
    nc = bacc.Bacc("TRN2", target_bir_lowering=False, debug=False,
                   enable_asserts=True, num_devices=NCORES)

    mT_in = nc.dram_tensor("mT_in", [P, JT, NS], F8, kind="ExternalInput")
    hT_in = nc.dram_tensor("hT_in", [P, JT, F], BF16, kind="ExternalInput")
    w1_in = nc.dram_tensor("w1_in", [F, F], F32, kind="ExternalInput")
    b1_in = nc.dram_tensor("b1_in", [F], F32, kind="ExternalInput")
    w2_in = nc.dram_tensor("w2_in", [F, F], F32, kind="ExternalInput")
    b2_in = nc.dram_tensor("b2_in", [F], F32, kind="ExternalInput")
    wc_in = nc.dram_tensor("wc_in", [C, F], F32, kind="ExternalInput")
    bc_in = nc.dram_tensor("bc_in", [C], F32, kind="ExternalInput")
    out_l = nc.dram_tensor("out_l", [G_LOCAL, C], F32, kind="ExternalOutput")

    with tile.TileContext(nc) as tc, ExitStack() as stack:
        consts = stack.enter_context(tc.tile_pool(name="consts", bufs=1))
        big = stack.enter_context(tc.tile_pool(name="big", bufs=1))
        dram = stack.enter_context(tc.tile_pool(name="dram", bufs=1, space="DRAM"))

        ident_bf = consts.tile([P, P], BF16)
        make_identity(nc, ident_bf)
        ident_f32 = consts.tile([P, P], F32)
        make_identity(nc, ident_f32)

        # ---- small constants -------------------------------------------
        ones_row = consts.tile([1, P], BF16)
        nc.vector.memset(ones_row[:], 1.0)
        ones_row8_f32 = consts.tile([1, G_LOCAL], F32)
        nc.vector.memset(ones_row8_f32[:], 1.0)
        b1_row = consts.tile([1, F], BF16)
        nc.gpsimd.dma_start(b1_row[:], b1_in.ap()[None, :])
        b2_row = consts.tile([1, F], BF16)
        nc.gpsimd.dma_start(b2_row[:], b2_in.ap()[None, :])
        bc_row = consts.tile([1, C], F32)
        nc.sync.dma_start(bc_row[:], bc_in.ap()[None, :])

        # w1T/w2T (transposed weights, bf16), wcT (f32)
        w1T = consts.tile([P, F], BF16)
        w2T = consts.tile([P, F], BF16)
        wcT = consts.tile([P, C], F32)
        with tc.tile_pool(name="wtmp", bufs=2) as wtmp, \
             tc.tile_pool(name="wpsum", bufs=2, space="PSUM") as wpsum:
            for w_in, wT in ((w1_in, w1T), (w2_in, w2T)):
                wf = wtmp.tile([F, F], F32, tag="wf")
                nc.sync.dma_start(wf[:], w_in.ap())
                wb = wtmp.tile([F, F], BF16, tag="wb")
                nc.vector.tensor_copy(wb[:], wf[:])
                ps = wpsum.tile([P, F], BF16, tag="wps")
                nc.tensor.transpose(ps[:], wb[:], ident_bf[:])
                nc.any.tensor_copy(wT[:], ps[:])
            wcf = wtmp.tile([C, F], F32, tag="wcf")
            nc.sync.dma_start(wcf[:], wc_in.ap())
            pc = wpsum.tile([P, C], F32, tag="wcps")
            nc.tensor.transpose(pc[:], wcf[:], ident_f32[:C, :C])
            nc.any.tensor_copy(wcT[:], pc[:])

        for _rep in range(reps):
            _emit_pipeline(
                nc, tc, consts, big, dram,
                mT_in, hT_in, out_l,
                ident_bf, ident_f32, ones_row, ones_row8_f32,
                b1_row, b2_row, bc_row, w1T, w2T, wcT,
            )

    nc.compile()
    return nc


def _emit_pipeline(nc, tc, consts, big, dram, mT_in, hT_in, out_l,
                   ident_bf, ident_f32, ones_row, ones_row8_f32,
                   b1_row, b2_row, bc_row, w1T, w2T, wcT):
    # ---- resident tensors ------------------------------------------
    mT = big.tile([P, JT, NS], F8, tag="mT", name="mT")          # [j_in_tile, jt, i]
    hT = big.tile([P, JT, F], BF16, tag="hT", name="hT")         # [e, jt, j_in_tile]
    msg_r = big.tile([P, JT, F], BF16, tag="msg_r", name="msg_r")   # relu, unscaled
    msg_f8 = big.tile([P, JT, F + 1], F8, tag="msg", name="msg")    # 64*sc*msg1 | 64
    ones_bf_scr = consts.tile([P, JT], BF16, tag="ones_scr", name="ones_scr")
    nc.vector.memset(ones_bf_scr[:], MSG_SCALE)
    nc.vector.tensor_copy(msg_f8[:, :, F], ones_bf_scr[:])
    cd_acc = big.tile([P, JT], F32, tag="cd_acc", name="cd_acc")

    # ---- phase A: mT + hT DMA; cd partials; msg1 relu (unscaled) ----
    nc.sync.dma_start(hT[:], hT_in.ap())
    KC = 8  # j-tiles per mT DMA chunk
    with tc.tile_pool(name="cdscratch", bufs=2) as cds, \
         tc.tile_pool(name="mpsum", bufs=4, space="PSUM") as mpsum:
        for k in range(JT // KC):
            nc.sync.dma_start(mT[:, k * KC:(k + 1) * KC, :],
                              mT_in.ap()[:, k * KC:(k + 1) * KC, :])
            # msg1 pre-relu staging first (PE matmuls + DVE copies) so the
            # PE->DVE pipeline stays hot; relu rides the post-collective
            # scale op (relu(s*z) = s*relu(z))
            for jt in range(k * KC, (k + 1) * KC):
                mps = mpsum.tile([P, F], F32, tag="mps", name="mps")
                nc.tensor.matmul(mps[:], hT[:, jt, :], w1T[:], start=True, stop=False)
                nc.tensor.matmul(mps[:], ones_row[:], b1_row[:], start=False, stop=True)
                nc.vector.tensor_copy(msg_r[:, jt, :], mps[:])
            for jt in range(k * KC, (k + 1) * KC):
                # cd partial: DVE reduce (~1.07us) / ACT accum-copy (~0.73us);
                # ACT gets the larger share so both drain together
                if jt % 3 == 0:
                    nc.vector.reduce_sum(
                        out=cd_acc[:, jt:jt + 1], in_=mT[:, jt, :],
                        axis=mybir.AxisListType.X)
                else:
                    scr = cds.tile([P, NS], F8, tag="cds", name="cds")
                    nc.scalar.activation(
                        scr[:], mT[:, jt, :],
                        mybir.ActivationFunctionType.Copy,
                        accum_out=cd_acc[:, jt:jt + 1])

    # ---- column-degree collectives ---------------------------------
    cd_part = dram.tile([N], F32, tag="cd_part", name="cd_part")
    cd_loc = dram.tile([NS], F32, tag="cd_loc", name="cd_loc")
    cd_full = dram.tile([N], F32, tag="cd_full", name="cd_full", addr_space="Shared")
    nc.sync.dma_start(cd_part[:].rearrange("(t p) -> p t", p=P), cd_acc[:])
    nc.gpsimd.collective_compute(
        "ReduceScatter", mybir.AluOpType.add,
        replica_groups=[list(range(NCORES))],
        ins=[cd_part.opt()], outs=[cd_loc.opt()],
    )
    nc.gpsimd.collective_compute(
        "AllGather", mybir.AluOpType.bypass,
        replica_groups=[list(range(NCORES))],
        ins=[cd_loc.opt()], outs=[cd_full.opt()],
    )
    cd_full_sb = consts.tile([P, JT], F32, tag="cdf_sb", name="cdf_sb")
    nc.sync.dma_start(cd_full_sb[:], cd_full[:].rearrange("(t p) -> p t", p=P))
    cd_loc_sb = consts.tile([P, IB], F32, tag="cdl_sb", name="cdl_sb")
    nc.sync.dma_start(cd_loc_sb[:], cd_loc[:].rearrange("(b p) -> p b", p=P))

    s_c64 = consts.tile([P, JT], F32, tag="s_c64", name="s_c64")
    nc.scalar.sqrt(s_c64[:], cd_full_sb[:])
    nc.vector.reciprocal(s_c64[:], s_c64[:])
    nc.vector.tensor_scalar_mul(s_c64[:], s_c64[:], MSG_SCALE)
    s_c_loc64 = consts.tile([P, IB], F32, tag="s_c_loc64", name="s_c_loc64")
    nc.scalar.sqrt(s_c_loc64[:], cd_loc_sb[:])
    nc.vector.reciprocal(s_c_loc64[:], s_c_loc64[:])
    nc.vector.tensor_scalar_mul(s_c_loc64[:], s_c_loc64[:], MSG_SCALE)

    # ---- phase C: t1*64 = m @ [64*s_c*msg1|64]; scaling pipelined ---
    s_r = consts.tile([P, IB], F32, tag="s_r", name="s_r")
    s_r64 = consts.tile([P, IB], F32, tag="s_r64", name="s_r64")
    h1_bf = consts.tile([P, IB, F], BF16, tag="h1_bf", name="h1_bf")
    with tc.tile_pool(name="c_psum", bufs=1, space="PSUM") as cpsum:
        pt1 = [cpsum.tile([P, F + 1], F32, tag=f"t1_{b}", name=f"t1_{b}")
               for b in range(IB)]
        def emit_scale(jt):
            # msg = relu(s_c64 * z) into fp8, split across ACT/DVE/gpsimd
            if jt % 3 == 0:
                nc.scalar.activation(
                    msg_f8[:, jt, 0:F], msg_r[:, jt, :],
                    mybir.ActivationFunctionType.Relu,
                    scale=s_c64[:, jt:jt + 1],
                )
            elif jt % 3 == 1:
                nc.vector.tensor_scalar(
                    out=msg_f8[:, jt, 0:F], in0=msg_r[:, jt, :],
                    scalar1=s_c64[:, jt:jt + 1], scalar2=0.0,
                    op0=mybir.AluOpType.mult, op1=mybir.AluOpType.max)
            else:
                nc.gpsimd.tensor_scalar(
                    out=msg_f8[:, jt, 0:F], in0=msg_r[:, jt, :],
                    scalar1=s_c64[:, jt:jt + 1], scalar2=0.0,
                    op0=mybir.AluOpType.mult, op1=mybir.AluOpType.max)

        # all scales first (3-way engine split, ~8us wall) so the PE
        # DoubleRow stream then runs back-to-back at full p-state
        for jt in range(JT):
            emit_scale(jt)
        if USE_DOUBLE_ROW:
            for tp in range(JT // 2):
                for b in range(IB):
                    nc.tensor.matmul(
                        pt1[b][:],
                        mT[:, 2 * tp:2 * tp + 2, b * P:(b + 1) * P],
                        msg_f8[:, 2 * tp:2 * tp + 2, :],
                        start=(tp == 0), stop=(tp == JT // 2 - 1),
                        perf_mode=mybir.MatmulPerfMode.DoubleRow,
                    )
        else:
            for jt in range(JT):
                for b in range(IB):
                    nc.tensor.matmul(
                        pt1[b][:], mT[:, jt, b * P:(b + 1) * P], msg_f8[:, jt, :],
                        start=(jt == 0), stop=(jt == JT - 1),
                    )
        # s_r = rsqrt(rd), rd = col F / 64;  h1 = (s_r/64) * t1*64
        for b in range(IB):
            nc.vector.tensor_scalar_mul(s_r[:, b:b + 1], pt1[b][:, F:F + 1],
                                        1.0 / MSG_SCALE)
        nc.scalar.sqrt(s_r[:], s_r[:])
        nc.vector.reciprocal(s_r[:], s_r[:])
        nc.vector.tensor_scalar_mul(s_r64[:], s_r[:], 1.0 / MSG_SCALE)
        for b in range(IB):
            nc.scalar.activation(
                h1_bf[:, b, :], pt1[b][:, 0:F],
                mybir.ActivationFunctionType.Copy,
                scale=s_r64[:, b:b + 1],
            )

    # ---- phase D: msg2 = 64*sc*relu(h1 W2^T + b2) as fp8, AllGather -
    msg2_sb = consts.tile([P, IB, F], F8, tag="msg2", name="msg2")
    with tc.tile_pool(name="d_work", bufs=2) as dwork, \
         tc.tile_pool(name="d_psum", bufs=2, space="PSUM") as dpsum, \
         tc.tile_pool(name="d_tpsum", bufs=2, space="PSUM") as dtpsum:
        for b in range(IB):
            tps = dtpsum.tile([P, P], BF16, tag="dtps", name="dtps")
            nc.tensor.transpose(tps[:], h1_bf[:, b, :], ident_bf[:])
            h1T = dwork.tile([P, F], BF16, tag="h1T", name="h1T")
            nc.any.tensor_copy(h1T[:], tps[:])
            ps = dpsum.tile([P, F], F32, tag="dps", name="dps")
            nc.tensor.matmul(ps[:], h1T[:], w2T[:], start=True, stop=False)
            nc.tensor.matmul(ps[:], ones_row[:], b2_row[:], start=False, stop=True)
            nc.scalar.activation(
                msg2_sb[:, b, :], ps[:],
                mybir.ActivationFunctionType.Relu,
                scale=s_c_loc64[:, b:b + 1],
            )

    msg2_loc = dram.tile([NS, F], F8, tag="m2l", name="m2l")
    msg2_full = dram.tile([N, F], F8, tag="m2f", name="m2f", addr_space="Shared")
    nc.sync.dma_start(
        msg2_loc[:].rearrange("(b p) g -> p b g", p=P), msg2_sb[:])
    nc.gpsimd.collective_compute(
        "AllGather", mybir.AluOpType.bypass,
        replica_groups=[list(range(NCORES))],
        ins=[msg2_loc.opt()], outs=[msg2_full.opt()],
    )
    # s_r/64 as a [1, NS] row via a dram round-trip + gpsimd partition
    # broadcast — emitted after the AllGather so it can't delay its issue;
    # runs during the collective, consumed at the end of phase E
    srd = dram.tile([NS], F32, tag="srd", name="srd")
    nc.sync.dma_start(srd[:].rearrange("(b p) -> p b", p=P), s_r64[:])
    s_r_row = consts.tile([1, NS], F32, tag="s_r_row", name="s_r_row")
    nc.sync.dma_start(s_r_row[:], srd[:][None, :])
    srb_sb = consts.tile([P, NS], F32, tag="srb_sb", name="srb_sb")
    nc.gpsimd.partition_broadcast(srb_sb[:], s_r_row[:])

    # ---- phase E: h2^T = (s_r/64) * (msg2^T m)^T, msg-stationary;
    #      readback chunks interleaved with the matmuls that consume them
    HNS = NS // 2
    RB = 8  # j-tiles per msg2_full readback chunk
    m2f_sb = big.tile([P, JT, F], F8, tag="m2f_sb", name="m2f_sb")
    h2T = consts.tile([P, NS], F32, tag="h2T", name="h2T")
    with tc.tile_pool(name="e_psum", bufs=1, space="PSUM") as epsum:
        pe = [epsum.tile([P, HNS], F32, tag=f"t2_{h}", name=f"t2_{h}")
              for h in range(2)]
        for k in range(JT // RB):
            nc.sync.dma_start(
                m2f_sb[:, k * RB:(k + 1) * RB, :],
                msg2_full[:].rearrange("(t p) g -> p t g", p=P)[:, k * RB:(k + 1) * RB, :])
            if USE_DOUBLE_ROW:
                for tp in range(k * RB // 2, (k + 1) * RB // 2):
                    for h in range(2):
                        nc.tensor.matmul(
                            pe[h][:],
                            m2f_sb[:, 2 * tp:2 * tp + 2, :],
                            mT[:, 2 * tp:2 * tp + 2, h * HNS:(h + 1) * HNS],
                            start=(tp == 0), stop=(tp == JT // 2 - 1),
                            perf_mode=mybir.MatmulPerfMode.DoubleRow,
                        )
            else:
                for jt in range(k * RB, (k + 1) * RB):
                    for h in range(2):
                        nc.tensor.matmul(
                            pe[h][:], m2f_sb[:, jt, :],
                            mT[:, jt, h * HNS:(h + 1) * HNS],
                            start=(jt == 0), stop=(jt == JT - 1),
                        )
        for h in range(2):
            nc.vector.scalar_tensor_tensor(
                out=h2T[:, h * HNS:(h + 1) * HNS],
                in0=pe[h][:], scalar=1.0, in1=srb_sb[:, h * HNS:(h + 1) * HNS],
                op0=mybir.AluOpType.mult, op1=mybir.AluOpType.mult,
            )

    # ---- phase F: segment max + classifier -------------------------
    pooledT = consts.tile([P, G_LOCAL], F32, tag="pooledT", name="pooledT")
    out_sb = consts.tile([G_LOCAL, C], F32, tag="out_sb", name="out_sb")
    nc.vector.reduce_max(
        out=pooledT[:], in_=h2T[:].rearrange("p (g q) -> p g q", q=P),
        axis=mybir.AxisListType.X)
    with tc.tile_pool(name="cls_psum", bufs=1, space="PSUM") as clspsum:
        cps = clspsum.tile([G_LOCAL, C], F32, tag="cls", name="cls")
        nc.tensor.matmul(cps[:], pooledT[:], wcT[:], start=True, stop=False)
        nc.tensor.matmul(cps[:], ones_row8_f32[:], bc_row[:],
                         start=False, stop=True)
        nc.vector.tensor_copy(out_sb[:], cps[:])
    nc.sync.dma_start(out_l.ap(), out_sb[:])


def _get_nc():
    if "nc" not in _CACHE:
        _CACHE["nc"] = _build()
    return _CACHE["nc"]


def make_in_maps(inputs):
    m = np.asarray(inputs["m"], dtype=np.float32)
    x = np.asarray(inputs["x"]).astype(np.int64)
    emb = np.asarray(inputs["emb"], dtype=np.float32)
    w1 = np.ascontiguousarray(np.asarray(inputs["w1"], dtype=np.float32))
    b1 = np.ascontiguousarray(np.asarray(inputs["b1"], dtype=np.float32))
    w2 = np.ascontiguousarray(np.asarray(inputs["w2"], dtype=np.float32))
    b2 = np.ascontiguousarray(np.asarray(inputs["b2"], dtype=np.float32))
    wc = np.ascontiguousarray(np.asarray(inputs["wc"], dtype=np.float32))
    bc = np.ascontiguousarray(np.asarray(inputs["bc"], dtype=np.float32))

    # host-side layout prep: embedding gather + per-core transposes
    h = emb[x]                                   # (N, F) float32
    hT = np.ascontiguousarray(
        h.reshape(JT, P, F).transpose(2, 0, 1)).astype(BF16_NP)  # [e, t, p]
    m8 = m.astype(M_NP_DTYPE)                    # fp8 cast once
    m8v = m8.reshape(N, JT, P)                   # [i_glob, t, p]

    in_maps = []
    for k in range(NCORES):
        blk = m8v[k * NS:(k + 1) * NS]           # [i_loc, t, p]
        mT = np.ascontiguousarray(blk.transpose(2, 1, 0))  # [p, t, i_loc]
        in_maps.append({
            "mT_in": mT, "hT_in": hT,
            "w1_in": w1, "b1_in": b1, "w2_in": w2, "b2_in": b2,
            "wc_in": wc, "bc_in": bc,
        })
    return in_maps


def kernel(**inputs):
    nc = _get_nc()
    in_maps = make_in_maps(inputs)
    res = bass_utils.run_bass_kernel_spmd(
        nc, in_maps, core_ids=list(range(NCORES)))
    out = np.concatenate([res.results[k]["out_l"] for k in range(NCORES)], axis=0)
    return out.astype(np.float32)
